# revision 1
# baseline (speedup 1.0000x reference)
"""MoE layer (top-2 routing, SwiGLU experts) on 8 TRN2 NeuronCores.

Strategy (expert-parallel, matching the sharding hint):
  - Host computes the router (logits -> top-2 -> softmax weights) in f64
    numpy. This is the dispatch decision of the all-to-all; it is ~0.05%
    of the FLOPs. The min gap between the 2nd and 3rd logit is ~1.7e-4,
    so f64 routing agrees with the fp32 reference's selection.
  - Core e receives the tokens routed to expert e (gathered, transposed,
    zero-padded to a static capacity C), expert e's weights
    (pre-transposed on host), and the per-token combine weight.
  - Each core runs the expert FFN: g = x@WgT, u = x@WuT, h = silu(g)*u,
    y = (h@WdT) * combine, all matmuls on the PE array in float32r
    (full-rate fp32, ~1.5e-4 matmul rel err).
  - Host scatter-adds each expert's scaled output rows into the full
    [T, H] output (the combine of the all-to-all).

Kernel layout per core (C = token capacity, chunked along tokens; chunks
grouped so weight stripes stream once per group):
  pass1 (per group, per i-tile, per chunk): g/u accumulate over 8
    h-tiles in PSUM (f32r matmuls, weights stationary, tokens moving),
    silu on ACT written into hh, in-place multiply on DVE ->
    hh[i] in SBUF, [i-part, c-free]. Wg/Wu are host-repacked into
    contiguous per-i-tile stripes for DMA efficiency.
  pass2 (per group, per 128-token tile): y accumulates over 22 i-tiles
    with stationary hh tiles and moving resident-WdT rows -> PSUM
    [c-part, 1024], then ACT copy with per-partition combine scale ->
    SBUF -> DRAM, token-major [C, H].
Measured: ~300 us on-device per invocation (all 8 cores in parallel),
~85% of the PE bf16-rate roofline for the padded sparse FLOPs, at fp32r
precision (end-to-end L2 rel err vs fp32 reference ~2.5e-4).
"""

import sys

if "/opt/trn_rl_repo" not in sys.path:
    sys.path.insert(0, "/opt/trn_rl_repo")

import numpy as np

B, S, H, I, E = 2, 2048, 1024, 2816, 8
T = B * S
HT = H // 128   # 8 h-tiles
IT = I // 128   # 22 i-tiles
TOP_K = 2

_PROG_CACHE = {}


def _split_waits(nc):
    """This walrus build rejects >1 sync wait per instruction; move extra
    waits onto standalone event-sem instructions on the issuing engine.
    For HWDGE DMAs the enqueue happens at engine-execution time, so a
    preceding engine-stream wait still gates the transfer."""
    import concourse.mybir as mybir

    for f in nc.m.functions:
        for blk in f.blocks:
            out = []
            for inst in blk.instructions:
                si = inst.sync_info
                if si is None or len(si.on_wait) <= 1:
                    out.append(inst)
                    continue
                waits = list(si.on_wait)
                for k, w in enumerate(waits[:-1]):
                    ev = mybir.InstEventSemaphore(name=f"{inst.name}_ws{k}")
                    ev.engine = inst.engine
                    ev.sync_info = mybir.SyncInfo(on_wait=[w], on_update=[])
                    out.append(ev)
                while len(si.on_wait) > 1:
                    si.on_wait.pop(0)
                out.append(inst)
            blk.instructions = out


CHUNK_MODE = "mixed"   # "mixed": 512s + tail; "384": all-384 chunks
WBUFS = 5              # wg/wu stripe prefetch depth
GROUP_CAP = 640        # max hh width per chunk-group
OUTBUFS = 1            # out staging depth
PASS_FILTER = None     # None | "p1" | "p2"  (diagnostics only)


def _chunks_of(C):
    """Split C (multiple of 128) into matmul-N chunks, each a multiple of
    128 with 256 <= cn <= 512 (f32r runs full-rate at N >= 256)."""
    out = []
    c0 = 0
    rem = C
    if CHUNK_MODE == "384" and C % 384 == 0:
        while rem > 0:
            out.append((c0, 384))
            c0 += 384
            rem -= 384
        return out
    while rem > 0:
        if rem > 512 and rem < 768:
            cn = rem - 256 if rem - 256 <= 512 else 384
        else:
            cn = min(512, rem)
        out.append((c0, cn))
        c0 += cn
        rem -= cn
    return out


def _build_program(C, repeat=1, bench=False):
    import concourse.bass as bass
    import concourse.mybir as mybir
    from concourse.tile import TileContext

    dt = mybir.dt
    f32 = dt.float32
    f32r = dt.float32r
    Silu = mybir.ActivationFunctionType.Silu
    CT = C // 128
    chunks = _chunks_of(C)

    nc = bass.Bass()
    if bench:
        # timing-only build: big tensors live in internal DRAM (no host
        # transfer); only a tiny dummy output is external
        xT = nc.dram_tensor("xT", [H, C], f32r)
        wg = nc.dram_tensor("wg", [IT, 128, H], f32r)
        wu = nc.dram_tensor("wu", [IT, 128, H], f32r)
        wd = nc.dram_tensor("wd", [I, H], f32r)
        ce = nc.dram_tensor("ce", [128, CT], f32)
        y = nc.dram_tensor("y", [C, H], f32)
        dummy = nc.declare_dram_parameter("bench_out", [128, 4], f32, isOutput=True)
    else:
        xT = nc.declare_dram_parameter("xT", [H, C], f32r, isOutput=False)
        wg = nc.declare_dram_parameter("wg", [IT, 128, H], f32r, isOutput=False)
        wu = nc.declare_dram_parameter("wu", [IT, 128, H], f32r, isOutput=False)
        wd = nc.declare_dram_parameter("wd", [I, H], f32r, isOutput=False)
        ce = nc.declare_dram_parameter("ce", [128, CT], f32, isOutput=False)
        y = nc.declare_dram_parameter("y", [C, H], f32, isOutput=True)

    wd_r = wd.rearrange("(it p) hd -> p it hd", p=128)
    xT_r = xT.rearrange("(ht p) c -> p ht c", p=128)

    with TileContext(nc) as tc:
        with (
            tc.tile_pool(name="resident", bufs=1) as resident,
            tc.tile_pool(name="wstripe", bufs=WBUFS) as wstripe,
            tc.tile_pool(name="xtp", bufs=1) as xtpool,
            tc.tile_pool(name="hh", bufs=1) as hhpool,
            tc.tile_pool(name="outp", bufs=OUTBUFS) as outp,
            tc.tile_pool(name="ps1", bufs=2, space="PSUM") as ps1,
            tc.tile_pool(name="ps2", bufs=2, space="PSUM") as ps2,
        ):
            if bench:
                # zero-fill internal tensors so timing data is clean fp
                zt = outp.tile([128, H], f32, tag="out")
                nc.vector.memset(zt[:, :], 0.0)
                ztr = zt[:, :].bitcast(f32r)

                def zfill(t, rows, cols):
                    for r in range(0, rows, 128):
                        for c in range(0, cols, H):
                            w = min(H, cols - c)
                            nc.sync.dma_start(
                                out=t[r:r + 128, c:c + w], in_=ztr[:, :w]
                            )

                for i in range(IT):
                    nc.sync.dma_start(out=wg[i, :, :], in_=ztr[:, :H])
                    nc.sync.dma_start(out=wu[i, :, :], in_=ztr[:, :H])
                zfill(wd, I, H)
                zfill(xT, H, C)
                nc.sync.dma_start(out=ce[:, :], in_=zt[:, :CT])

            # Resident tensors
            ce_sb = resident.tile([128, CT], f32)
            nc.sync.dma_start(out=ce_sb[:, :], in_=ce[:, :])
            wd_sb = resident.tile([128, IT, H], f32r)
            for i in range(IT):
                nc.sync.dma_start(out=wd_sb[:, i, :], in_=wd_r[:, i, :])

            # group chunks so weight stripes stream once per group while
            # hh (sized to max group width) + resident wd fit in SBUF
            groups = []
            for c0, cn in chunks:
                if groups and sum(c[1] for c in groups[-1]) + cn <= GROUP_CAP:
                    groups[-1].append((c0, cn))
                else:
                    groups.append([(c0, cn)])
            hh_w = max(sum(c[1] for c in grp) for grp in groups)

            def body():
                for grp in groups:
                    g0 = grp[0][0]
                    gw = sum(c[1] for c in grp)
                    hh = hhpool.tile([128, IT, hh_w], f32r, tag="hh")
                    xt_sb = xtpool.tile([128, HT, hh_w], f32r, tag="xt")
                    nc.sync.dma_start(
                        out=xt_sb[:, :, :gw], in_=xT_r[:, :, g0:g0 + gw]
                    )
                    # ---- pass 1: hh = silu(x@WgT) * (x@WuT) ----
                    for i in range(IT) if PASS_FILTER != "p2" else ():
                        wgt = wstripe.tile([128, HT, 128], f32r, tag="wg")
                        wut = wstripe.tile([128, HT, 128], f32r, tag="wu")
                        nc.sync.dma_start(
                            out=wgt[:, :, :].rearrange("p ht c -> p (ht c)"),
                            in_=wg[i, :, :],
                        )
                        nc.sync.dma_start(
                            out=wut[:, :, :].rearrange("p ht c -> p (ht c)"),
                            in_=wu[i, :, :],
                        )
                        for c0, cn in grp:
                            g_ps = ps1.tile([128, 512], f32, tag="g")
                            u_ps = ps1.tile([128, 512], f32, tag="u")
                            for h in range(HT):
                                nc.tensor.matmul(
                                    g_ps[:, :cn],
                                    wgt[:, h, :],
                                    xt_sb[:, h, c0 - g0:c0 - g0 + cn],
                                    start=(h == 0),
                                    stop=(h == HT - 1),
                                )
                                nc.tensor.matmul(
                                    u_ps[:, :cn],
                                    wut[:, h, :],
                                    xt_sb[:, h, c0 - g0:c0 - g0 + cn],
                                    start=(h == 0),
                                    stop=(h == HT - 1),
                                )
                            hslice = hh[:, i, c0 - g0:c0 - g0 + cn]
                            nc.scalar.activation(hslice, g_ps[:, :cn], Silu)
                            nc.vector.tensor_mul(hslice, hslice, u_ps[:, :cn])
                    # ---- pass 2: y = (hh @ WdT) * combine ----
                    for c0, cn in grp if PASS_FILTER != "p1" else ():
                        for ci in range(cn // 128):
                            y_ps = ps2.tile([128, H], f32, tag="y")
                            cs = c0 + ci * 128
                            hs = c0 - g0 + ci * 128
                            for i in range(IT):
                                for nh in range(2):
                                    nc.tensor.matmul(
                                        y_ps[:, nh * 512:(nh + 1) * 512],
                                        hh[:, i, hs:hs + 128],
                                        wd_sb[:, i, nh * 512:(nh + 1) * 512],
                                        start=(i == 0),
                                        stop=(i == IT - 1),
                                    )
                            out_sb = outp.tile([128, H], f32, tag="out")
                            nc.scalar.activation(
                                out_sb[:, :],
                                y_ps[:, :],
                                mybir.ActivationFunctionType.Copy,
                                scale=ce_sb[:, cs // 128:cs // 128 + 1],
                            )
                            nc.sync.dma_start(
                                out=y[cs:cs + 128, :], in_=out_sb[:, :]
                            )

            if repeat == 1:
                body()
            else:
                with tc.For_i(0, repeat, 1):
                    body()

            if bench:
                nc.sync.dma_start(out=dummy[:, :], in_=ce_sb[:, :4])

    _split_waits(nc)
    return nc


def _route(xf, router_w):
    """Host-side router: replicate reference's top-2 + softmax in f64."""
    logits = xf.astype(np.float64) @ router_w.astype(np.float64).T  # [T, E]
    # stable argsort of negated logits == top_k tie-break (lower idx first)
    order = np.argsort(-logits, axis=1, kind="stable")[:, :TOP_K]  # [T, 2]
    top_vals = np.take_along_axis(logits, order, axis=1)
    ex = np.exp(top_vals - top_vals[:, :1])
    top_w = ex / ex.sum(axis=1, keepdims=True)  # [T, 2]
    return order.astype(np.int64), top_w


def kernel(x, router_w, Wg, Wu, Wd):
    from concourse.bass_utils import run_bass_kernel_spmd

    in_dtype = x.dtype
    xf = np.ascontiguousarray(x.reshape(T, H), dtype=np.float32)
    top_idx, top_w = _route(xf, np.asarray(router_w, dtype=np.float32))

    # per-expert token lists
    ids = []
    wts = []
    for e in range(E):
        sel = np.nonzero(top_idx == e)
        ids.append(sel[0])
        wts.append(top_w[sel[0], sel[1]].astype(np.float32))
    counts = np.array([len(a) for a in ids])
    C = max(256, int(np.ceil(counts.max() / 128)) * 128)

    def stripe_pack(w):
        # [I, H] -> [IT, 128p(h%128), HT*128(i-col)] contiguous stripes
        a = np.asarray(w, np.float32).reshape(IT, 128, HT, 128)
        return np.ascontiguousarray(a.transpose(0, 3, 2, 1).reshape(IT, 128, H))

    in_maps = []
    for e in range(E):
        n_e = counts[e]
        xT_e = np.zeros((H, C), dtype=np.float32)
        xT_e[:, :n_e] = xf[ids[e]].T
        ce_col = np.zeros(C, dtype=np.float32)
        ce_col[:n_e] = wts[e]
        ce_e = np.ascontiguousarray(ce_col.reshape(C // 128, 128).T)
        in_maps.append(
            {
                "xT": xT_e,
                "wg": stripe_pack(Wg[e]),
                "wu": stripe_pack(Wu[e]),
                "wd": np.ascontiguousarray(np.asarray(Wd[e], np.float32).T),
                "ce": ce_e,
            }
        )

    nc = _PROG_CACHE.get(C)
    if nc is None:
        nc = _build_program(C)
        _PROG_CACHE[C] = nc

    res = run_bass_kernel_spmd(nc, in_maps, list(range(E)))

    out = np.zeros((T, H), dtype=np.float32)
    for e in range(E):
        n_e = counts[e]
        np.add.at(out, ids[e], res.results[e]["y"][:n_e])
    return out.reshape(B, S, H).astype(in_dtype, copy=False)



# revision 2
# speedup vs baseline: 1.2457x; 1.2457x over previous
"""MoE layer (top-2 routing, SwiGLU experts) on 8 TRN2 NeuronCores.

Strategy (expert-parallel, matching the sharding hint):
  - Host computes the router (logits -> top-2 -> softmax weights) in f64
    numpy. This is the dispatch decision of the all-to-all; it is ~0.05%
    of the FLOPs. The min gap between the 2nd and 3rd logit is ~1.7e-4,
    so f64 routing agrees with the fp32 reference's selection.
  - Core e receives the tokens routed to expert e (gathered, transposed,
    zero-padded to a static capacity C), expert e's weights
    (pre-transposed on host), and the per-token combine weight.
  - Each core runs the expert FFN: g = x@WgT, u = x@WuT, h = silu(g)*u,
    y = (h@WdT) * combine. All matmul operands are bf16 (same PE rate as
    float32r on TRN2 -- 1 cycle/moving-row -- but half the DMA bytes, and
    SBUF then fits hh + x for the full token capacity, so each weight
    stripe is DMA'd exactly once per invocation).
  - Host scatter-adds each expert's scaled output rows into the full
    [T, H] output (the combine of the all-to-all).

Kernel layout per core (C = token capacity, chunks of <=512 tokens):
  pass1 (per i-tile, per chunk): g/u accumulate over 8 h-tiles in PSUM,
    silu on ACT -> hh bf16 in SBUF [i-part, token-free], in-place
    multiply by u on DVE. x arrives as 3 chunk-sized DMAs so the first
    chunk's matmuls start ~3us earlier than a monolithic load allows.
  pass2 (per 128-token tile): y accumulates over 22 i-tiles with
    stationary hh tiles and moving resident-WdT rows -> PSUM
    [c-part, 1024], then ACT copy with per-partition combine scale ->
    SBUF -> DRAM, token-major [C, H].
Measured (robust slope protocol, R=1000 vs 3000): ~264-270 us per
invocation on-device vs ~310 us for the float32r 2-group predecessor.
End-to-end L2 rel err vs fp32 reference ~4.1e-3 (gate 2e-2).
"""

import sys

if "/opt/trn_rl_repo" not in sys.path:
    sys.path.insert(0, "/opt/trn_rl_repo")

import numpy as np
import ml_dtypes

B, S, H, I, E = 2, 2048, 1024, 2816, 8
T = B * S
HT = H // 128   # 8 h-tiles
IT = I // 128   # 22 i-tiles
TOP_K = 2

_PROG_CACHE = {}


def _split_waits(nc):
    """This walrus build rejects >1 sync wait per instruction; move extra
    waits onto standalone event-sem instructions on the issuing engine.
    For HWDGE DMAs the enqueue happens at engine-execution time, so a
    preceding engine-stream wait still gates the transfer."""
    import concourse.mybir as mybir

    for f in nc.m.functions:
        for blk in f.blocks:
            out = []
            for inst in blk.instructions:
                si = inst.sync_info
                if si is None or len(si.on_wait) <= 1:
                    out.append(inst)
                    continue
                waits = list(si.on_wait)
                for k, w in enumerate(waits[:-1]):
                    ev = mybir.InstEventSemaphore(name=f"{inst.name}_ws{k}")
                    ev.engine = inst.engine
                    ev.sync_info = mybir.SyncInfo(on_wait=[w], on_update=[])
                    out.append(ev)
                while len(si.on_wait) > 1:
                    si.on_wait.pop(0)
                out.append(inst)
            blk.instructions = out


CHUNK_MODE = "mixed"   # "mixed": 512s + tail; "384": all-384 chunks
WBUFS = 5              # wg/wu stripe prefetch depth
GROUP_CAP = 1152       # bf16: full C fits one group (weights stream once)
OUTBUFS = 1            # out staging depth
PASS_FILTER = None     # None | "p1" | "p2"  (diagnostics only)
XT_SPLIT = 3           # xt DMA segments (first chunk lands first)
OUTQ = "sp"            # engine queue for output DMAs: "sp" | "act"


def _chunks_of(C):
    """Split C (multiple of 128) into matmul-N chunks, each a multiple of
    128 with 256 <= cn <= 512 (f32r runs full-rate at N >= 256)."""
    out = []
    c0 = 0
    rem = C
    if CHUNK_MODE == "384" and C % 384 == 0:
        while rem > 0:
            out.append((c0, 384))
            c0 += 384
            rem -= 384
        return out
    while rem > 0:
        if rem > 512 and rem < 768:
            cn = rem - 256 if rem - 256 <= 512 else 384
        else:
            cn = min(512, rem)
        out.append((c0, cn))
        c0 += cn
        rem -= cn
    return out


def _build_program(C, repeat=1, bench=False):
    import concourse.bass as bass
    import concourse.mybir as mybir
    from concourse.tile import TileContext

    dt = mybir.dt
    f32 = dt.float32
    bf16 = dt.bfloat16
    Silu = mybir.ActivationFunctionType.Silu
    CT = C // 128
    chunks = _chunks_of(C)

    nc = bass.Bass()
    if bench:
        # timing-only build: big tensors live in internal DRAM (no host
        # transfer); only a tiny dummy output is external
        xT = nc.dram_tensor("xT", [H, C], bf16)
        wg = nc.dram_tensor("wg", [IT, 128, H], bf16)
        wu = nc.dram_tensor("wu", [IT, 128, H], bf16)
        wd = nc.dram_tensor("wd", [I, H], bf16)
        ce = nc.dram_tensor("ce", [128, CT], f32)
        y = nc.dram_tensor("y", [C, H], f32)
        dummy = nc.declare_dram_parameter("bench_out", [128, 4], f32, isOutput=True)
    else:
        xT = nc.declare_dram_parameter("xT", [H, C], bf16, isOutput=False)
        wg = nc.declare_dram_parameter("wg", [IT, 128, H], bf16, isOutput=False)
        wu = nc.declare_dram_parameter("wu", [IT, 128, H], bf16, isOutput=False)
        wd = nc.declare_dram_parameter("wd", [I, H], bf16, isOutput=False)
        ce = nc.declare_dram_parameter("ce", [128, CT], f32, isOutput=False)
        y = nc.declare_dram_parameter("y", [C, H], f32, isOutput=True)

    wd_r = wd.rearrange("(it p) hd -> p it hd", p=128)
    xT_r = xT.rearrange("(ht p) c -> p ht c", p=128)

    with TileContext(nc) as tc:
        with (
            tc.tile_pool(name="resident", bufs=1) as resident,
            tc.tile_pool(name="wstripe", bufs=WBUFS) as wstripe,
            tc.tile_pool(name="xtp", bufs=1) as xtpool,
            tc.tile_pool(name="hh", bufs=1) as hhpool,
            tc.tile_pool(name="outp", bufs=OUTBUFS) as outp,
            tc.tile_pool(name="ps1", bufs=2, space="PSUM") as ps1,
            tc.tile_pool(name="ps2", bufs=2, space="PSUM") as ps2,
        ):
            if bench:
                # zero-fill internal tensors so timing data is clean fp
                zt = outp.tile([128, H], f32, tag="out")
                nc.vector.memset(zt[:, :], 0.0)
                ztr = zt[:, :].bitcast(bf16)

                def zfill(t, rows, cols):
                    for r in range(0, rows, 128):
                        for c in range(0, cols, 2 * H):
                            w = min(2 * H, cols - c)
                            nc.sync.dma_start(
                                out=t[r:r + 128, c:c + w], in_=ztr[:, :w]
                            )

                for i in range(IT):
                    nc.sync.dma_start(out=wg[i, :, :], in_=ztr[:, :H])
                    nc.sync.dma_start(out=wu[i, :, :], in_=ztr[:, :H])
                zfill(wd, I, H)
                zfill(xT, H, C)
                nc.sync.dma_start(out=ce[:, :], in_=zt[:, :CT])

            # Resident tensors
            ce_sb = resident.tile([128, CT], f32)
            nc.sync.dma_start(out=ce_sb[:, :], in_=ce[:, :])
            wd_sb = resident.tile([128, IT, H], bf16)
            for i in range(IT):
                nc.sync.dma_start(out=wd_sb[:, i, :], in_=wd_r[:, i, :])

            # group chunks so weight stripes stream once per group while
            # hh (sized to max group width) + resident wd fit in SBUF
            groups = []
            for c0, cn in chunks:
                if groups and sum(c[1] for c in groups[-1]) + cn <= GROUP_CAP:
                    groups[-1].append((c0, cn))
                else:
                    groups.append([(c0, cn)])
            hh_w = max(sum(c[1] for c in grp) for grp in groups)

            def body():
                for grp in groups:
                    g0 = grp[0][0]
                    gw = sum(c[1] for c in grp)
                    hh = hhpool.tile([128, IT, hh_w], bf16, tag="hh")
                    xt_sb = xtpool.tile([128, HT, hh_w], bf16, tag="xt")
                    if XT_SPLIT <= 1:
                        nc.sync.dma_start(
                            out=xt_sb[:, :, :gw], in_=xT_r[:, :, g0:g0 + gw]
                        )
                    else:
                        # chunk-aligned segments; first chunk lands first
                        for c0, cn in grp[:XT_SPLIT - 1]:
                            nc.sync.dma_start(
                                out=xt_sb[:, :, c0 - g0:c0 - g0 + cn],
                                in_=xT_r[:, :, c0:c0 + cn],
                            )
                        c0r = grp[XT_SPLIT - 1][0]
                        if c0r < g0 + gw:
                            nc.sync.dma_start(
                                out=xt_sb[:, :, c0r - g0:gw],
                                in_=xT_r[:, :, c0r:g0 + gw],
                            )
                    # ---- pass 1: hh = silu(x@WgT) * (x@WuT) ----
                    for i in range(IT) if PASS_FILTER != "p2" else ():
                        wgt = wstripe.tile([128, HT, 128], bf16, tag="wg")
                        wut = wstripe.tile([128, HT, 128], bf16, tag="wu")
                        nc.sync.dma_start(
                            out=wgt[:, :, :].rearrange("p ht c -> p (ht c)"),
                            in_=wg[i, :, :],
                        )
                        nc.sync.dma_start(
                            out=wut[:, :, :].rearrange("p ht c -> p (ht c)"),
                            in_=wu[i, :, :],
                        )
                        for c0, cn in grp:
                            g_ps = ps1.tile([128, 512], f32, tag="g")
                            u_ps = ps1.tile([128, 512], f32, tag="u")
                            for h in range(HT):
                                nc.tensor.matmul(
                                    g_ps[:, :cn],
                                    wgt[:, h, :],
                                    xt_sb[:, h, c0 - g0:c0 - g0 + cn],
                                    start=(h == 0),
                                    stop=(h == HT - 1),
                                )
                                nc.tensor.matmul(
                                    u_ps[:, :cn],
                                    wut[:, h, :],
                                    xt_sb[:, h, c0 - g0:c0 - g0 + cn],
                                    start=(h == 0),
                                    stop=(h == HT - 1),
                                )
                            hslice = hh[:, i, c0 - g0:c0 - g0 + cn]
                            nc.scalar.activation(hslice, g_ps[:, :cn], Silu)
                            nc.vector.tensor_mul(hslice, hslice, u_ps[:, :cn])
                    # ---- pass 2: y = (hh @ WdT) * combine ----
                    for c0, cn in grp if PASS_FILTER != "p1" else ():
                        for ci in range(cn // 128):
                            y_ps = ps2.tile([128, H], f32, tag="y")
                            cs = c0 + ci * 128
                            hs = c0 - g0 + ci * 128
                            for i in range(IT):
                                for nh in range(2):
                                    nc.tensor.matmul(
                                        y_ps[:, nh * 512:(nh + 1) * 512],
                                        hh[:, i, hs:hs + 128],
                                        wd_sb[:, i, nh * 512:(nh + 1) * 512],
                                        start=(i == 0),
                                        stop=(i == IT - 1),
                                    )
                            out_sb = outp.tile([128, H], f32, tag="out")
                            nc.scalar.activation(
                                out_sb[:, :],
                                y_ps[:, :],
                                mybir.ActivationFunctionType.Copy,
                                scale=ce_sb[:, cs // 128:cs // 128 + 1],
                            )
                            (nc.scalar if OUTQ == "act" else nc.sync).dma_start(
                                out=y[cs:cs + 128, :], in_=out_sb[:, :]
                            )

            if repeat == 1:
                body()
            else:
                with tc.For_i(0, repeat, 1):
                    body()

            if bench:
                nc.sync.dma_start(out=dummy[:, :], in_=ce_sb[:, :4])

    _split_waits(nc)
    return nc


def _route(xf, router_w):
    """Host-side router: replicate reference's top-2 + softmax in f64."""
    logits = xf.astype(np.float64) @ router_w.astype(np.float64).T  # [T, E]
    # stable argsort of negated logits == top_k tie-break (lower idx first)
    order = np.argsort(-logits, axis=1, kind="stable")[:, :TOP_K]  # [T, 2]
    top_vals = np.take_along_axis(logits, order, axis=1)
    ex = np.exp(top_vals - top_vals[:, :1])
    top_w = ex / ex.sum(axis=1, keepdims=True)  # [T, 2]
    return order.astype(np.int64), top_w


def kernel(x, router_w, Wg, Wu, Wd):
    from concourse.bass_utils import run_bass_kernel_spmd

    in_dtype = x.dtype
    xf = np.ascontiguousarray(x.reshape(T, H), dtype=np.float32)
    top_idx, top_w = _route(xf, np.asarray(router_w, dtype=np.float32))

    # per-expert token lists
    ids = []
    wts = []
    for e in range(E):
        sel = np.nonzero(top_idx == e)
        ids.append(sel[0])
        wts.append(top_w[sel[0], sel[1]].astype(np.float32))
    counts = np.array([len(a) for a in ids])
    C = max(256, int(np.ceil(counts.max() / 128)) * 128)

    bf = ml_dtypes.bfloat16

    def stripe_pack(w):
        # [I, H] -> [IT, 128p(h%128), HT*128(i-col)] contiguous stripes
        a = np.asarray(w, np.float32).reshape(IT, 128, HT, 128)
        return np.ascontiguousarray(
            a.transpose(0, 3, 2, 1).reshape(IT, 128, H).astype(bf)
        )

    in_maps = []
    for e in range(E):
        n_e = counts[e]
        xT_e = np.zeros((H, C), dtype=bf)
        xT_e[:, :n_e] = xf[ids[e]].T.astype(bf)
        ce_col = np.zeros(C, dtype=np.float32)
        ce_col[:n_e] = wts[e]
        ce_e = np.ascontiguousarray(ce_col.reshape(C // 128, 128).T)
        in_maps.append(
            {
                "xT": xT_e,
                "wg": stripe_pack(Wg[e]),
                "wu": stripe_pack(Wu[e]),
                "wd": np.ascontiguousarray(np.asarray(Wd[e], np.float32).T.astype(bf)),
                "ce": ce_e,
            }
        )

    nc = _PROG_CACHE.get(C)
    if nc is None:
        nc = _build_program(C)
        _PROG_CACHE[C] = nc

    res = run_bass_kernel_spmd(nc, in_maps, list(range(E)))

    out = np.zeros((T, H), dtype=np.float32)
    for e in range(E):
        n_e = counts[e]
        np.add.at(out, ids[e], res.results[e]["y"][:n_e])
    return out.reshape(B, S, H).astype(in_dtype, copy=False)

